# revision 4
# baseline (speedup 1.0000x reference)
"""BiLSTM-CRF Trainium2 kernel (8 NeuronCores, SPMD).

Strategy: fully symmetric SPMD program; direction differences live in
per-core input data. Core 0 runs the forward LSTM on `sentence`, core 1
runs the same program on the reversed sentence with the backward weights
(yielding the backward LSTM), cores 2-7 run on zeros. Each core computes
its tag-projection partial featsT [50, T] (core 1 in reversed time
order), an AllGather shares all partials, and every core runs the
identical Viterbi forward DP. The host backtracks the path from core 0's
backpointers.

The recurrent matvec uses h-stationary column-tiled matmuls (4
concurrent 128x32 PE column groups streaming W_hh as the moving
operand), gate rows are reshaped to a [32,128] tile via
partition-strided DMAs, transposed on the PE (accumulating the
precomputed input projection xg in PSUM), and the LSTM pointwise math
runs on ACT/DVE in a [128, 32] layout.
"""
import sys
sys.path.insert(0, '/opt/trn_rl_repo')
import time
import numpy as np

import concourse.bass as bass
import concourse.mybir as mybir
import concourse.tile as tile
from concourse import bacc
from concourse.bass import ds
from concourse.bass_utils import run_bass_kernel_spmd
from concourse.masks import make_identity

FP = mybir.dt.float32
H = 1024
NKC = 8                  # 128-wide h chunks
G4 = 4096                # gate dim (permuted order i,f,o,g)
IN = 2048
NIC = IN // 128          # 16 input chunks
TAGS = 48
K = TAGS + 2
START, STOP = TAGS, TAGS + 1
NEG = -10000.0
NCORES = 8

_PERM = np.concatenate([np.arange(0, H), np.arange(H, 2 * H),
                        np.arange(3 * H, 4 * H), np.arange(2 * H, 3 * H)])


# ---------------------------------------------------------------- host packing

def _pack_whh(w_hh):
    w = w_hh[_PERM]                                  # [4096, 1024]
    wt = w.T.reshape(NKC, 128, G4)                   # [kc, p, g]
    return np.ascontiguousarray(wt.transpose(1, 0, 2).reshape(128, NKC * G4)).astype(np.float32)


def _pack_wih(w_ih):
    w = w_ih[_PERM]                                  # [4096, 2048]
    return np.ascontiguousarray(w.T.reshape(NIC, 128, G4)).astype(np.float32)


def _pack_vec8(v):
    return np.ascontiguousarray(v.reshape(NKC, 128).T).astype(np.float32)


def _pack_wout(w_half):
    # [50, 1024] -> [128, 8*64] ; chunk kc at cols kc*64, first 50 cols used
    out = np.zeros((128, NKC * 64), np.float32)
    for kc in range(NKC):
        out[:, kc * 64:kc * 64 + K] = w_half[:, kc * 128:(kc + 1) * 128].T
    return out


# ---------------------------------------------------------------- program

def build_program(T):
    NB = T // 128                 # recurrence blocks
    NSUP = T // 512               # projection super-blocks
    nc = bacc.Bacc()

    x_in = nc.dram_tensor("x", [T, IN], FP, kind="ExternalInput")
    wih_in = nc.dram_tensor("wih", [NIC, 128, G4], FP, kind="ExternalInput")
    whh_in = nc.dram_tensor("whh", [128, NKC * G4], FP, kind="ExternalInput")
    bias_in = nc.dram_tensor("bias", [1, G4], FP, kind="ExternalInput")
    h0_in = nc.dram_tensor("h0", [128, 8], FP, kind="ExternalInput")
    c0_in = nc.dram_tensor("c0", [128, 8], FP, kind="ExternalInput")
    wout_in = nc.dram_tensor("wout", [128, NKC * 64], FP, kind="ExternalInput")
    bout_in = nc.dram_tensor("bout", [64, 1], FP, kind="ExternalInput")
    trans_in = nc.dram_tensor("trans", [K, K], FP, kind="ExternalInput")
    initfv_in = nc.dram_tensor("initfv", [1, K], FP, kind="ExternalInput")

    bptr_out = nc.dram_tensor("bptr", [K, T * 8], mybir.dt.uint16, kind="ExternalOutput")
    fv_out = nc.dram_tensor("fv", [1, K], FP, kind="ExternalOutput")

    xg_dram = nc.dram_tensor("xg_scratch", [T, G4], FP)
    cc_in = nc.dram_tensor("cc_in", [K, T], FP)
    cc_out = nc.dram_tensor("cc_out", [NCORES * K, T], FP, addr_space="Shared")

    with tile.TileContext(nc) as tc:
        with tc.tile_pool(name="glob", bufs=1) as glob:
            ident = glob.tile([128, 128], FP, tag="ident")
            make_identity(nc, ident[:])
            feats = glob.tile([64, T], FP, tag="feats")

            # ---------------- phase 1: xg = x @ w_ih.T + bias -> xg_dram
            with tc.tile_pool(name="p1", bufs=1) as p1, \
                 tc.tile_pool(name="p1w", bufs=2) as p1w, \
                 tc.tile_pool(name="p1ps", bufs=2, space="PSUM") as p1ps:
                bias_sb = p1.tile([1, G4], FP, tag="bias")
                nc.sync.dma_start(bias_sb[:], bias_in[:])
                bias_full = p1.tile([128, G4], FP, tag="biasfull")
                nc.gpsimd.partition_broadcast(bias_full[:], bias_sb[:])

                def proj_body(sup):
                    xts = []
                    for tb in range(4):
                        xblk = p1w.tile([128, IN], FP, tag="xblk")
                        nc.sync.dma_start(xblk[:], x_in[ds(sup * 512 + tb * 128, 128), :])
                        xt = p1w.tile([128, IN], FP, tag=f"xt{tb}")
                        for kc2 in range(0, NIC, 4):
                            tp = p1ps.tile([128, 512], FP, tag="tp")
                            for j in range(4):
                                nc.tensor.matmul(
                                    tp[:, j * 128:(j + 1) * 128],
                                    xblk[:, (kc2 + j) * 128:(kc2 + j + 1) * 128],
                                    ident[:], is_transpose=True,
                                    start=True, stop=True, skip_group_check=True)
                            nc.vector.tensor_copy(xt[:, kc2 * 128:(kc2 + 4) * 128], tp[:])
                        xts.append(xt)
                    for ng in range(8):
                        wts = []
                        for kc in range(NIC):
                            wt = p1w.tile([128, 512], FP, tag=f"w{kc}")
                            nc.sync.dma_start(wt[:], wih_in[kc, :, ng * 512:(ng + 1) * 512])
                            wts.append(wt)
                        for tb in range(4):
                            ps = p1ps.tile([128, 512], FP, tag="mm")
                            for kc in range(NIC):
                                nc.tensor.matmul(
                                    ps[:], xts[tb][:, kc * 128:(kc + 1) * 128], wts[kc][:],
                                    start=(kc == 0), stop=(kc == NIC - 1),
                                    skip_group_check=True)
                            ev = p1w.tile([128, 512], FP, tag="ev")
                            nc.vector.tensor_tensor(
                                ev[:], ps[:],
                                bias_full[:, ng * 512:(ng + 1) * 512],
                                mybir.AluOpType.add)
                            nc.sync.dma_start(
                                xg_dram[ds(sup * 512 + tb * 128, 128), ng * 512:(ng + 1) * 512],
                                ev[:])

                tc.For_i_unrolled(0, NSUP, 1, proj_body, max_unroll=1)

            # ---------------- phase 2: recurrence + featsT partial
            with tc.tile_pool(name="p2", bufs=1) as p2, \
                 tc.tile_pool(name="p2s", bufs=3) as p2s, \
                 tc.tile_pool(name="p2ps", bufs=2, space="PSUM") as p2ps:
                W = p2.tile([128, NKC * G4], FP, tag="W")
                nc.sync.dma_start(W[:], whh_in[:])
                woutsb = p2.tile([128, NKC * 64], FP, tag="wout")
                nc.sync.dma_start(woutsb[:], wout_in[:])
                hstate = p2.tile([128, 8], FP, tag="hstate")
                nc.sync.dma_start(hstate[:], h0_in[:])
                cstate = p2.tile([128, 8], FP, tag="cstate")
                nc.sync.dma_start(cstate[:], c0_in[:])
                hbuf = p2.tile([128, H], FP, tag="hbuf")
                hb3 = hbuf.rearrange("p (kc i) -> p kc i", i=128)

                def rec_body(b):
                    for i in range(128):
                        xg32 = p2s.tile([32, 128], FP, tag="xg32")
                        nc.sync.dma_start(
                            xg32[:],
                            xg_dram[ds(b * 128 + i, 1), :].rearrange("o (c f) -> (o c) f", f=128))
                        if i == 0:
                            hcol = lambda kc: hstate[:, kc:kc + 1]
                        else:
                            hcol = lambda kc, i=i: hb3[:, kc, i - 1:i]
                        _emit_step(nc, p2s, p2ps, W, hb3, cstate, xg32, hcol, i, ident)
                    # feats partial for this block
                    fp = p2ps.tile([64, 128], FP, tag="fp")
                    for kc in range(NKC):
                        nc.tensor.matmul(
                            fp[0:K, :], woutsb[:, kc * 64:kc * 64 + K],
                            hbuf[:, kc * 128:(kc + 1) * 128],
                            start=(kc == 0), stop=(kc == NKC - 1),
                            skip_group_check=True)
                    nc.vector.tensor_copy(feats[0:K, ds(b * 128, 128)], fp[0:K, :])
                    nc.vector.tensor_copy(hstate[:], hb3[:, :, 127])

                tc.For_i_unrolled(0, NB, 1, rec_body, max_unroll=1)

                bout_sb = p2.tile([64, 1], FP, tag="bout")
                nc.sync.dma_start(bout_sb[:], bout_in[:])
                nc.vector.tensor_scalar_add(feats[0:K, :], feats[0:K, :], bout_sb[0:K, 0:1])

            # ---------------- phase 3: share partials, Viterbi DP
            fout = nc.dram_tensor("f_loc", [K, T], FP)
            with tc.tile_pool(name="p3", bufs=1) as p3, \
                 tc.tile_pool(name="p3s", bufs=3) as p3s, \
                 tc.tile_pool(name="p3ps", bufs=1, space="PSUM") as p3ps:
                nc.sync.dma_start(cc_in[:], feats[0:K, :])
                nc.gpsimd.collective_compute(
                    "AllGather", mybir.AluOpType.bypass,
                    replica_groups=[list(range(NCORES))],
                    ins=[cc_in[:]], outs=[cc_out[:]])
                A = p3.tile([64, T], FP, tag="A")
                B = p3.tile([64, T], FP, tag="B")
                nc.sync.dma_start(A[0:K, :], cc_out[0:K, :])
                nc.sync.dma_start(B[0:K, :], cc_out[K:2 * K, :])
                trans_sb = p3.tile([64, 64], FP, tag="trans")
                nc.sync.dma_start(trans_sb[0:K, 0:K], trans_in[:])
                bptr = p3.tile([64, T * 8], mybir.dt.uint16, tag="bptr")
                fv_sb = p3.tile([1, 64], FP, tag="fvsb")
                nc.sync.dma_start(fv_sb[0:1, 0:K], initfv_in[:])
                ones50 = p3.tile([1, 64], FP, tag="ones")
                nc.vector.memset(ones50[0:1, 0:K], 1.0)

                def vit_body(t):
                    scps = p3ps.tile([64, 64], FP, tag="scps")
                    nc.tensor.matmul(scps[0:K, 0:K], ones50[0:1, 0:K],
                                     fv_sb[0:1, 0:K],
                                     start=True, stop=False, skip_group_check=True)
                    nc.tensor.matmul(scps[0:K, 0:K], ident[0:K, 0:K],
                                     trans_sb[0:K, 0:K],
                                     start=False, stop=True, skip_group_check=True)
                    sc = p3s.tile([64, 64], FP, tag="sc")
                    nc.vector.tensor_copy(sc[0:K, 0:K], scps[0:K, 0:K])
                    mx = p3s.tile([64, 8], FP, tag="mx")
                    nc.vector.max(mx[0:K, :], sc[0:K, 0:K])
                    nc.vector.max_index(bptr[0:K, ds(t * 8, 8)], mx[0:K, :], sc[0:K, 0:K])
                    fa = p3s.tile([64, 1], FP, tag="fa")
                    nc.vector.tensor_tensor(fa[0:K, :], mx[0:K, 0:1],
                                            A[0:K, ds(t, 1)], mybir.AluOpType.add)
                    fb = p3s.tile([64, 1], FP, tag="fb")
                    nc.vector.tensor_tensor(fb[0:K, :], fa[0:K, :],
                                            B[0:K, ds(T - 1 - t, 1)], mybir.AluOpType.add)
                    fvps = p3ps.tile([1, 64], FP, tag="fvrow")
                    nc.tensor.matmul(fvps[0:1, 0:K], fb[0:K, 0:1], ident[0:K, 0:K],
                                     is_transpose=True, start=True, stop=True,
                                     skip_group_check=True)
                    nc.vector.tensor_copy(fv_sb[0:1, 0:K], fvps[0:1, 0:K])

                tc.For_i_unrolled(0, T, 1, vit_body, max_unroll=16)

                nc.sync.dma_start(fv_out[:], fv_sb[0:1, 0:K])
                nc.sync.dma_start(bptr_out[:], bptr[0:K, :])

    nc.compile()
    return nc


def _emit_step(nc, sb, pp, W, hb3, cstate, xg32, hcol, i_col, ident):
    rows4 = [None, None]
    for nh in range(2):
        ps = pp.tile([128, 512], FP, tag="mv")
        for kc in range(NKC):
            lhsT = hcol(kc)
            for q in range(4):
                off = kc * G4 + q * 1024 + nh * 512
                nc.tensor.matmul(
                    ps[32 * q:32 * q + 1, :], lhsT, W[:, off:off + 512],
                    start=(kc == 0), stop=(kc == NKC - 1),
                    tile_position=(0, 32 * q), skip_group_check=True)
        r4 = sb.tile([128, 512], FP, tag=f"rows4_{nh}")
        nc.vector.tensor_copy(r4[:], ps[:])
        rows4[nh] = r4
    gates32 = sb.tile([32, 128], FP, tag="gates32")
    dst_r = gates32.rearrange("(q cc) f -> cc q f", cc=8)
    for nh in range(2):
        src_r = rows4[nh].rearrange("(q r) (cc f) -> q r cc f", r=32, cc=4)
        for cc in range(4):
            nc.sync.dma_start(dst_r[4 * nh + cc], src_r[:, 0, cc, :])
    gps = pp.tile([128, 32], FP, tag="gt")
    nc.tensor.matmul(gps[:], xg32[:], ident[0:32, 0:32], is_transpose=True,
                     start=True, stop=False, skip_group_check=True)
    nc.tensor.matmul(gps[:], gates32[:], ident[0:32, 0:32], is_transpose=True,
                     start=False, stop=True, skip_group_check=True)
    S = sb.tile([128, 24], FP, tag="S")
    tg = sb.tile([128, 8], FP, tag="tg")
    nc.scalar.activation(S[:], gps[:, 0:24], mybir.ActivationFunctionType.Sigmoid)
    nc.scalar.activation(tg[:], gps[:, 24:32], mybir.ActivationFunctionType.Tanh)
    t1 = sb.tile([128, 8], FP, tag="t1")
    t2 = sb.tile([128, 8], FP, tag="t2")
    nc.vector.tensor_mul(t1[:], S[:, 8:16], cstate[:])
    nc.vector.tensor_mul(t2[:], S[:, 0:8], tg[:])
    nc.vector.tensor_add(cstate[:], t1[:], t2[:])
    tc_ = sb.tile([128, 8], FP, tag="tc")
    nc.scalar.activation(tc_[:], cstate[:], mybir.ActivationFunctionType.Tanh)
    nc.vector.tensor_mul(hb3[:, :, i_col], S[:, 16:24], tc_[:])


# ---------------------------------------------------------------- host driver

_prog_cache = {}


def _get_program(T):
    if T not in _prog_cache:
        _prog_cache[T] = build_program(T)
    return _prog_cache[T]


def _make_in_maps(inputs, T):
    s = np.asarray(inputs["sentence"], np.float32)
    zeros_x = np.zeros_like(s)
    trans = np.asarray(inputs["transitions"], np.float32)
    initfv = np.full((1, K), NEG, np.float32)
    initfv[0, START] = 0.0
    bout_pad = np.zeros((64, 1), np.float32)
    bout_pad[:K, 0] = np.asarray(inputs["b_out"], np.float32)
    zb = np.zeros((64, 1), np.float32)

    def core_map(x, wih, whh, b, h0v, c0v, wout_half, bout):
        return {
            "x": np.ascontiguousarray(x),
            "wih": _pack_wih(wih),
            "whh": _pack_whh(whh),
            "bias": np.ascontiguousarray(b[_PERM][None, :]).astype(np.float32),
            "h0": _pack_vec8(h0v), "c0": _pack_vec8(c0v),
            "wout": _pack_wout(wout_half),
            "bout": bout,
            "trans": np.ascontiguousarray(trans),
            "initfv": initfv,
        }

    w_out = np.asarray(inputs["w_out"], np.float32)
    h0 = np.asarray(inputs["h0"], np.float32)
    c0 = np.asarray(inputs["c0"], np.float32)
    maps = [
        core_map(s, inputs["w_ih_f"], inputs["w_hh_f"], inputs["b_f"],
                 h0[0], c0[0], w_out[:, :H], bout_pad),
        core_map(s[::-1], inputs["w_ih_b"], inputs["w_hh_b"], inputs["b_b"],
                 h0[1], c0[1], w_out[:, H:], zb),
    ]
    zmap = core_map(zeros_x, np.zeros((G4, IN), np.float32),
                    np.zeros((G4, H), np.float32), np.zeros(G4, np.float32),
                    np.zeros(H, np.float32), np.zeros(H, np.float32),
                    np.zeros((K, H), np.float32), zb)
    for _ in range(2, NCORES):
        maps.append(zmap)
    return maps


def run_device(inputs, T=None, nc=None):
    T = T or np.asarray(inputs["sentence"]).shape[0]
    nc = nc or _get_program(T)
    maps = _make_in_maps(inputs, T)
    res = run_bass_kernel_spmd(nc, maps, core_ids=list(range(NCORES)))
    return res


def postprocess(res, inputs, T):
    trans = np.asarray(inputs["transitions"], np.float32)
    out0 = res.results[0]
    fv = out0["fv"][0]                       # [50]
    bptr = out0["bptr"].reshape(K, T, 8)[:, :, 0].astype(np.int64)  # [50, T]
    terminal = fv + trans[STOP]
    best = int(np.argmax(terminal))
    score = np.float32(terminal[best])
    path = np.zeros(T, np.int32)
    path[T - 1] = best
    tag = best
    for t in range(T - 1, 0, -1):
        tag = int(bptr[tag, t])
        path[t - 1] = tag
    return np.asarray(score, np.float32), path


def kernel(**inputs):
    T = np.asarray(inputs["sentence"]).shape[0]
    res = run_device(inputs, T)
    return postprocess(res, inputs, T)


if __name__ == "__main__":
    dat = np.load("/tmp/inputs.npz")
    inputs = {k: dat[k] for k in dat.files}
    t0 = time.time()
    score, path = kernel(**inputs)
    print("total wall", time.time() - t0)
    print("score", score, "path[:8]", path[:8])


# revision 5
# speedup vs baseline: 1.3051x; 1.3051x over previous
"""BiLSTM-CRF Trainium2 kernel (8 NeuronCores, SPMD).

Strategy: fully symmetric SPMD program; direction differences live in
per-core input data. Core 0 runs the forward LSTM on `sentence`, core 1
runs the same program on the reversed sentence with the backward weights
(yielding the backward LSTM), cores 2-7 run on zeros. Each core computes
its tag-projection partial featsT [50, T] (core 1 in reversed time
order), an AllGather shares all partials, and every core runs the
identical Viterbi forward DP. The host backtracks the path from core 0's
backpointers.

The recurrent matvec uses h-stationary column-tiled matmuls (4
concurrent 128x32 PE column groups streaming W_hh as the moving
operand), gate rows are reshaped to a [32,128] tile via
partition-strided DMAs, transposed on the PE (accumulating the
precomputed input projection xg in PSUM), and the LSTM pointwise math
runs on ACT/DVE in a [128, 32] layout.
"""
import sys
sys.path.insert(0, '/opt/trn_rl_repo')
import time
import numpy as np

import concourse.bass as bass
import concourse.mybir as mybir
import concourse.tile as tile
from concourse import bacc
from concourse.bass import ds
from concourse.bass_utils import run_bass_kernel_spmd
from concourse.masks import make_identity

FP = mybir.dt.float32
H = 1024
NKC = 8                  # 128-wide h chunks
G4 = 4096                # gate dim (permuted order i,f,o,g)
IN = 2048
NIC = IN // 128          # 16 input chunks
TAGS = 48
K = TAGS + 2
START, STOP = TAGS, TAGS + 1
NEG = -10000.0
NCORES = 8

_PERM = np.concatenate([np.arange(0, H), np.arange(H, 2 * H),
                        np.arange(3 * H, 4 * H), np.arange(2 * H, 3 * H)])


# ---------------------------------------------------------------- host packing

def _pack_whh(w_hh):
    w = w_hh[_PERM]                                  # [4096, 1024]
    wt = w.T.reshape(NKC, 128, G4)                   # [kc, p, g]
    return np.ascontiguousarray(wt.transpose(1, 0, 2).reshape(128, NKC * G4)).astype(np.float32)


def _pack_wih(w_ih):
    w = w_ih[_PERM]                                  # [4096, 2048]
    return np.ascontiguousarray(w.T.reshape(NIC, 128, G4)).astype(np.float32)


def _pack_vec8(v):
    return np.ascontiguousarray(v.reshape(NKC, 128).T).astype(np.float32)


def _pack_wout(w_half):
    # [50, 1024] -> [128, 8*64] ; chunk kc at cols kc*64, first 50 cols used
    out = np.zeros((128, NKC * 64), np.float32)
    for kc in range(NKC):
        out[:, kc * 64:kc * 64 + K] = w_half[:, kc * 128:(kc + 1) * 128].T
    return out


# ---------------------------------------------------------------- program

def build_program(T):
    NB = T // 128                 # recurrence blocks
    NSUP = T // 512               # projection super-blocks
    nc = bacc.Bacc()

    x_in = nc.dram_tensor("x", [T, IN], FP, kind="ExternalInput")
    wih_in = nc.dram_tensor("wih", [NIC, 128, G4], FP, kind="ExternalInput")
    whh_in = nc.dram_tensor("whh", [128, NKC * G4], FP, kind="ExternalInput")
    bias_in = nc.dram_tensor("bias", [1, G4], FP, kind="ExternalInput")
    h0_in = nc.dram_tensor("h0", [128, 8], FP, kind="ExternalInput")
    c0_in = nc.dram_tensor("c0", [128, 8], FP, kind="ExternalInput")
    wout_in = nc.dram_tensor("wout", [128, NKC * 64], FP, kind="ExternalInput")
    bout_in = nc.dram_tensor("bout", [64, 1], FP, kind="ExternalInput")
    trans_in = nc.dram_tensor("trans", [K, K], FP, kind="ExternalInput")
    initfv_in = nc.dram_tensor("initfv", [1, K], FP, kind="ExternalInput")

    bptr_out = nc.dram_tensor("bptr", [K, T * 8], mybir.dt.uint16, kind="ExternalOutput")
    fv_out = nc.dram_tensor("fv", [1, K], FP, kind="ExternalOutput")

    xg_dram = nc.dram_tensor("xg_scratch", [T, G4], FP)
    cc_in = nc.dram_tensor("cc_in", [K, T], FP)
    cc_out = nc.dram_tensor("cc_out", [NCORES * K, T], FP, addr_space="Shared")

    with tile.TileContext(nc) as tc:
        with tc.tile_pool(name="glob", bufs=1) as glob:
            ident = glob.tile([128, 128], FP, tag="ident")
            make_identity(nc, ident[:])
            feats = glob.tile([64, T], FP, tag="feats")

            # ---------------- phase 1: xg = x @ w_ih.T + bias -> xg_dram
            with tc.tile_pool(name="p1", bufs=1) as p1, \
                 tc.tile_pool(name="p1w", bufs=2) as p1w, \
                 tc.tile_pool(name="p1ps", bufs=2, space="PSUM") as p1ps:
                bias_sb = p1.tile([1, G4], FP, tag="bias")
                nc.sync.dma_start(bias_sb[:], bias_in[:])
                bias_full = p1.tile([128, G4], FP, tag="biasfull")
                nc.gpsimd.partition_broadcast(bias_full[:], bias_sb[:])

                def proj_body(sup):
                    xts = []
                    for tb in range(4):
                        xblk = p1w.tile([128, IN], FP, tag="xblk")
                        nc.sync.dma_start(xblk[:], x_in[ds(sup * 512 + tb * 128, 128), :])
                        xt = p1w.tile([128, IN], FP, tag=f"xt{tb}")
                        for kc2 in range(0, NIC, 4):
                            tp = p1ps.tile([128, 512], FP, tag="tp")
                            for j in range(4):
                                nc.tensor.matmul(
                                    tp[:, j * 128:(j + 1) * 128],
                                    xblk[:, (kc2 + j) * 128:(kc2 + j + 1) * 128],
                                    ident[:], is_transpose=True,
                                    start=True, stop=True, skip_group_check=True)
                            nc.vector.tensor_copy(xt[:, kc2 * 128:(kc2 + 4) * 128], tp[:])
                        xts.append(xt)
                    for ng in range(8):
                        wts = []
                        for kc in range(NIC):
                            wt = p1w.tile([128, 512], FP, tag=f"w{kc}")
                            nc.sync.dma_start(wt[:], wih_in[kc, :, ng * 512:(ng + 1) * 512])
                            wts.append(wt)
                        for tb in range(4):
                            ps = p1ps.tile([128, 512], FP, tag="mm")
                            for kc in range(NIC):
                                nc.tensor.matmul(
                                    ps[:], xts[tb][:, kc * 128:(kc + 1) * 128], wts[kc][:],
                                    start=(kc == 0), stop=(kc == NIC - 1),
                                    skip_group_check=True)
                            ev = p1w.tile([128, 512], FP, tag="ev")
                            nc.vector.tensor_tensor(
                                ev[:], ps[:],
                                bias_full[:, ng * 512:(ng + 1) * 512],
                                mybir.AluOpType.add)
                            nc.sync.dma_start(
                                xg_dram[ds(sup * 512 + tb * 128, 128), ng * 512:(ng + 1) * 512],
                                ev[:])

                tc.For_i_unrolled(0, NSUP, 1, proj_body, max_unroll=1)

            # ---------------- phase 2: recurrence + featsT partial
            with tc.tile_pool(name="p2", bufs=1) as p2, \
                 tc.tile_pool(name="p2s", bufs=3) as p2s, \
                 tc.tile_pool(name="p2ps", bufs=2, space="PSUM") as p2ps:
                W = p2.tile([128, NKC * G4], FP, tag="W")
                nc.sync.dma_start(W[:], whh_in[:])
                woutsb = p2.tile([128, NKC * 64], FP, tag="wout")
                nc.sync.dma_start(woutsb[:], wout_in[:])
                hstate = p2.tile([128, 8], FP, tag="hstate")
                nc.sync.dma_start(hstate[:], h0_in[:])
                cstate = p2.tile([128, 8], FP, tag="cstate")
                nc.sync.dma_start(cstate[:], c0_in[:])
                hbuf = p2.tile([128, H], FP, tag="hbuf")
                hb3 = hbuf.rearrange("p (kc i) -> p kc i", i=128)

                def rec_body(b):
                    for i in range(128):
                        xg32 = p2s.tile([32, 128], FP, tag="xg32")
                        nc.sync.dma_start(
                            xg32[:],
                            xg_dram[ds(b * 128 + i, 1), :].rearrange("o (c f) -> (o c) f", f=128))
                        if i == 0:
                            hcol = lambda kc: hstate[:, kc:kc + 1]
                        else:
                            hcol = lambda kc, i=i: hb3[:, kc, i - 1:i]
                        _emit_step(nc, p2s, p2ps, W, hb3, cstate, xg32, hcol, i, ident)
                    # feats partial for this block
                    fp = p2ps.tile([64, 128], FP, tag="fp")
                    for kc in range(NKC):
                        nc.tensor.matmul(
                            fp[0:K, :], woutsb[:, kc * 64:kc * 64 + K],
                            hbuf[:, kc * 128:(kc + 1) * 128],
                            start=(kc == 0), stop=(kc == NKC - 1),
                            skip_group_check=True)
                    nc.vector.tensor_copy(feats[0:K, ds(b * 128, 128)], fp[0:K, :])
                    nc.vector.tensor_copy(hstate[:], hb3[:, :, 127])

                tc.For_i_unrolled(0, NB, 1, rec_body, max_unroll=1)

                bout_sb = p2.tile([64, 1], FP, tag="bout")
                nc.sync.dma_start(bout_sb[:], bout_in[:])
                nc.vector.tensor_scalar_add(feats[0:K, :], feats[0:K, :], bout_sb[0:K, 0:1])

            # ---------------- phase 3: share partials, Viterbi DP
            fout = nc.dram_tensor("f_loc", [K, T], FP)
            with tc.tile_pool(name="p3", bufs=1) as p3, \
                 tc.tile_pool(name="p3s", bufs=3) as p3s, \
                 tc.tile_pool(name="p3ps", bufs=1, space="PSUM") as p3ps:
                nc.sync.dma_start(cc_in[:], feats[0:K, :])
                nc.gpsimd.collective_compute(
                    "AllGather", mybir.AluOpType.bypass,
                    replica_groups=[list(range(NCORES))],
                    ins=[cc_in[:]], outs=[cc_out[:]])
                A = p3.tile([64, T], FP, tag="A")
                B = p3.tile([64, T], FP, tag="B")
                nc.sync.dma_start(A[0:K, :], cc_out[0:K, :])
                nc.sync.dma_start(B[0:K, :], cc_out[K:2 * K, :])
                trans_sb = p3.tile([64, 64], FP, tag="trans")
                nc.sync.dma_start(trans_sb[0:K, 0:K], trans_in[:])
                bptr = p3.tile([64, T * 8], mybir.dt.uint16, tag="bptr")
                fv_sb = p3.tile([1, 64], FP, tag="fvsb")
                nc.sync.dma_start(fv_sb[0:1, 0:K], initfv_in[:])
                ones50 = p3.tile([1, 64], FP, tag="ones")
                nc.vector.memset(ones50[0:1, 0:K], 1.0)

                def vit_body(t):
                    scps = p3ps.tile([64, 64], FP, tag="scps")
                    nc.tensor.matmul(scps[0:K, 0:K], ones50[0:1, 0:K],
                                     fv_sb[0:1, 0:K],
                                     start=True, stop=False, skip_group_check=True)
                    nc.tensor.matmul(scps[0:K, 0:K], ident[0:K, 0:K],
                                     trans_sb[0:K, 0:K],
                                     start=False, stop=True, skip_group_check=True)
                    sc = p3s.tile([64, 64], FP, tag="sc")
                    nc.vector.tensor_copy(sc[0:K, 0:K], scps[0:K, 0:K])
                    mx = p3s.tile([64, 8], FP, tag="mx")
                    nc.vector.max(mx[0:K, :], sc[0:K, 0:K])
                    nc.vector.max_index(bptr[0:K, ds(t * 8, 8)], mx[0:K, :], sc[0:K, 0:K])
                    fa = p3s.tile([64, 1], FP, tag="fa")
                    nc.vector.tensor_tensor(fa[0:K, :], mx[0:K, 0:1],
                                            A[0:K, ds(t, 1)], mybir.AluOpType.add)
                    fb = p3s.tile([64, 1], FP, tag="fb")
                    nc.vector.tensor_tensor(fb[0:K, :], fa[0:K, :],
                                            B[0:K, ds(T - 1 - t, 1)], mybir.AluOpType.add)
                    fvps = p3ps.tile([1, 64], FP, tag="fvrow")
                    nc.tensor.matmul(fvps[0:1, 0:K], fb[0:K, 0:1], ident[0:K, 0:K],
                                     is_transpose=True, start=True, stop=True,
                                     skip_group_check=True)
                    nc.vector.tensor_copy(fv_sb[0:1, 0:K], fvps[0:1, 0:K])

                tc.For_i_unrolled(0, T, 1, vit_body, max_unroll=16)

                nc.sync.dma_start(fv_out[:], fv_sb[0:1, 0:K])
                nc.sync.dma_start(bptr_out[:], bptr[0:K, :])

    nc.compile()
    return nc


def _emit_step(nc, sb, pp, W, hb3, cstate, xg32, hcol, i_col, ident):
    rows4 = [None, None]
    for nh in range(2):
        ps = pp.tile([128, 512], FP, tag="mv")
        for kc in range(NKC):
            lhsT = hcol(kc)
            for q in range(4):
                off = kc * G4 + q * 1024 + nh * 512
                nc.tensor.matmul(
                    ps[32 * q:32 * q + 1, :], lhsT, W[:, off:off + 512],
                    start=(kc == 0), stop=(kc == NKC - 1),
                    tile_position=(0, 32 * q), skip_group_check=True)
        r4 = sb.tile([128, 512], FP, tag=f"rows4_{nh}")
        nc.vector.tensor_copy(r4[:], ps[:])
        rows4[nh] = r4
    gates32 = sb.tile([32, 128], FP, tag="gates32")
    dst_r = gates32.rearrange("(q cc) f -> cc q f", cc=8)
    for nh in range(2):
        src_r = rows4[nh].rearrange("(q r) (cc f) -> q r cc f", r=32, cc=4)
        for cc in range(4):
            nc.sync.dma_start(dst_r[4 * nh + cc], src_r[:, 0, cc, :])
    gps = pp.tile([128, 32], FP, tag="gt")
    nc.tensor.matmul(gps[:], xg32[:], ident[0:32, 0:32], is_transpose=True,
                     start=True, stop=False, skip_group_check=True)
    nc.tensor.matmul(gps[:], gates32[:], ident[0:32, 0:32], is_transpose=True,
                     start=False, stop=True, skip_group_check=True)
    S = sb.tile([128, 24], FP, tag="S")
    tg = sb.tile([128, 8], FP, tag="tg")
    nc.scalar.activation(S[:], gps[:, 0:24], mybir.ActivationFunctionType.Sigmoid)
    nc.scalar.activation(tg[:], gps[:, 24:32], mybir.ActivationFunctionType.Tanh)
    t1 = sb.tile([128, 8], FP, tag="t1")
    t2 = sb.tile([128, 8], FP, tag="t2")
    nc.vector.tensor_mul(t1[:], S[:, 8:16], cstate[:])
    nc.vector.tensor_mul(t2[:], S[:, 0:8], tg[:])
    nc.vector.tensor_add(cstate[:], t1[:], t2[:])
    tc_ = sb.tile([128, 8], FP, tag="tc")
    nc.scalar.activation(tc_[:], cstate[:], mybir.ActivationFunctionType.Tanh)
    nc.vector.tensor_mul(hb3[:, :, i_col], S[:, 16:24], tc_[:])


# ---------------------------------------------------------------- host driver

_prog_cache = {}


def _get_program(T):
    if T not in _prog_cache:
        _prog_cache[T] = build_program(T)
    return _prog_cache[T]


def _make_in_maps(inputs, T):
    inputs = {k: np.asarray(v) for k, v in inputs.items()}
    s = np.asarray(inputs["sentence"], np.float32)
    zeros_x = np.zeros_like(s)
    trans = np.asarray(inputs["transitions"], np.float32)
    initfv = np.full((1, K), NEG, np.float32)
    initfv[0, START] = 0.0
    bout_pad = np.zeros((64, 1), np.float32)
    bout_pad[:K, 0] = np.asarray(inputs["b_out"], np.float32)
    zb = np.zeros((64, 1), np.float32)

    def core_map(x, wih, whh, b, h0v, c0v, wout_half, bout):
        return {
            "x": np.ascontiguousarray(x),
            "wih": _pack_wih(wih),
            "whh": _pack_whh(whh),
            "bias": np.ascontiguousarray(b[_PERM][None, :]).astype(np.float32),
            "h0": _pack_vec8(h0v), "c0": _pack_vec8(c0v),
            "wout": _pack_wout(wout_half),
            "bout": bout,
            "trans": np.ascontiguousarray(trans),
            "initfv": initfv,
        }

    w_out = np.asarray(inputs["w_out"], np.float32)
    h0 = np.asarray(inputs["h0"], np.float32)
    c0 = np.asarray(inputs["c0"], np.float32)
    maps = [
        core_map(s, inputs["w_ih_f"], inputs["w_hh_f"], inputs["b_f"],
                 h0[0], c0[0], w_out[:, :H], bout_pad),
        core_map(s[::-1], inputs["w_ih_b"], inputs["w_hh_b"], inputs["b_b"],
                 h0[1], c0[1], w_out[:, H:], zb),
    ]
    zmap = core_map(zeros_x, np.zeros((G4, IN), np.float32),
                    np.zeros((G4, H), np.float32), np.zeros(G4, np.float32),
                    np.zeros(H, np.float32), np.zeros(H, np.float32),
                    np.zeros((K, H), np.float32), zb)
    for _ in range(2, NCORES):
        maps.append(zmap)
    return maps


def run_device(inputs, T=None, nc=None):
    T = T or np.asarray(inputs["sentence"]).shape[0]
    nc = nc or _get_program(T)
    maps = _make_in_maps(inputs, T)
    res = run_bass_kernel_spmd(nc, maps, core_ids=list(range(NCORES)))
    return res


def postprocess(res, inputs, T):
    trans = np.asarray(inputs["transitions"], np.float32)
    out0 = res.results[0]
    fv = out0["fv"][0]                       # [50]
    bptr = out0["bptr"].reshape(K, T, 8)[:, :, 0].astype(np.int64)  # [50, T]
    terminal = fv + trans[STOP]
    best = int(np.argmax(terminal))
    score = np.float32(terminal[best])
    path = np.zeros(T, np.int32)
    path[T - 1] = best
    tag = best
    for t in range(T - 1, 0, -1):
        tag = int(bptr[tag, t])
        path[t - 1] = tag
    return np.asarray(score, np.float32), path


def kernel(**inputs):
    T = np.asarray(inputs["sentence"]).shape[0]
    res = run_device(inputs, T)
    return postprocess(res, inputs, T)


if __name__ == "__main__":
    dat = np.load("/tmp/inputs.npz")
    inputs = {k: dat[k] for k in dat.files}
    t0 = time.time()
    score, path = kernel(**inputs)
    print("total wall", time.time() - t0)
    print("score", score, "path[:8]", path[:8])
